# revision 17
# baseline (speedup 1.0000x reference)
"""NT-Xent loss kernel for Trainium2 (8 NeuronCores, row-parallel).

Math: loss = mean_r [ log(sum_{k!=r} exp(sim[r,k]/T)) - pos_r/T ],
T=0.5, sim = reps @ reps.T with reps = l2norm rows of [z_i; z_j].

Device (per core c, rows slab = [1024c, 1024c+1024)):
  S[r] = sum_k exp(2 * sim[r,k])   (full row, including diagonal)
via bf16 matmul (PE) + fused exp/row-sum (ACT accum_out).

Host: normalize rows, cast to bf16 (exactly what PE consumes), compute
positives and the bf16 self-dot rr exactly, then
  loss_r = log(S[r] - exp(2*rr_r)) - 2*pos_r,  loss = mean_r loss_r.
No collectives needed: final mean over 8192 rows done on host.
"""

import os
import sys

import numpy as np

for _p in ("/opt/trn_rl_repo",):
    if _p not in sys.path and os.path.isdir(_p):
        sys.path.insert(0, _p)

import ml_dtypes

N = 4096
D = 256
TWO_N = 2 * N
N_CORES = 8
SLAB = TWO_N // N_CORES  # 1024 rows per core
M_TILES = SLAB // 128  # 8
N_CHUNK = 512
GROUP = 4 * N_CHUNK  # 2048 cols per ACT/exp group (4 PSUM banks)
N_GROUPS = TWO_N // GROUP  # 4

LAST_EXEC_NS = None
LAST_RESULTS = None

_cached_nc = {}


_SERIAL_ENGINES = {
    "EngineType.PE",
    "EngineType.Activation",
    "EngineType.DVE",
    "EngineType.Pool",
    "EngineType.SP",
}
_CHECKED_OPS = ("InstMatmult", "InstActivation", "InstTensorReduce", "InstDrain")


def _reduce_waits(nc, tc):
    """Walrus HW instruction structs hold a single sync-wait, but with
    optimize_sems disabled (inc-6505) Tile leaves transitively-redundant
    waits in place (e.g. a PE self-drain next to an ACT wait that already
    implies it, or the final SP drain waiting on every sem when the
    out-DMA's completion implies them all).

    Sound vector-clock transitive reduction over the finalized program:
    walk instructions in emission order (a valid topological order of the
    sync graph), track per-sem "value v reached => clock C held" and
    per-serial-queue completion clocks (compute/SP queues consume their
    FIFOs serially; sems only increase). A wait is droppable when the
    queue clock plus the instruction's remaining waits already imply it.
    DMACopy triggers are handled asynchronously: their sem updates carry
    the trigger-time clock plus the copy's own waits, and the SP queue
    clock does not absorb transfer completion.
    """
    import itertools

    import bass_rust

    # Execution order: inst_map insertion order is right for everything
    # except instructions created during post-schedule lowering (e.g. the
    # Ldweights split out of each matmul get late ids but execute paired
    # before their matmul). The tile stream has those in true order, so
    # splice it into the finalize-added pre/postamble at the position of
    # its first member.
    stream = [
        i for block in tc.ordered_instructions_by_block.values() for i in block
    ]
    stream_ids = {id(i) for i in stream}
    insts = []
    spliced = False
    for inst in nc.inst_map.values():
        if id(inst) in stream_ids:
            if not spliced:
                insts.extend(stream)
                spliced = True
        else:
            insts.append(inst)
    sem_hist = {}  # sem id -> [(cum value, clock dict)]
    sem_cum = {}
    queue_clock = {}

    def clock_at(sid, v):
        for cum, clk in sem_hist.get(sid, ()):  # short lists
            if cum >= v:
                return clk
        return None

    def join(dst, src):
        for k, v in src.items():
            if v > dst.get(k, 0):
                dst[k] = v

    for inst in insts:
        eng = str(getattr(inst, "engine", ""))
        si = inst.sync_info
        is_dma = type(inst).__name__ == "InstDMACopy"
        base = dict(queue_clock.get(eng, {})) if eng in _SERIAL_ENGINES else {}
        waits = list(si.on_wait) if si else []
        wclocks = []
        for w in waits:
            c = None
            if (
                w.sync_type == "semaphore"
                and w.wait_mode == "sem-ge-imm"
                and w.wait_value is not None
            ):
                c = clock_at(w.id, w.wait_value)
                if c is None and base.get(w.id, 0) >= w.wait_value:
                    c = {}
                if c is not None:
                    c = dict(c)
                    c[w.id] = max(c.get(w.id, 0), w.wait_value)
            wclocks.append(c)

        if len(waits) > 1 and all(c is not None for c in wclocks):
            n = len(waits)
            best = None
            for r in range(n):
                for comb in itertools.combinations(range(n), r):
                    acc = dict(base)
                    for i in comb:
                        join(acc, wclocks[i])
                    if all(
                        acc.get(w.id, 0) >= w.wait_value for w in waits
                    ):
                        best = comb
                        break
                if best is not None:
                    break
            if best is not None:
                kept = [waits[i] for i in best]
                inst.sync_info = bass_rust.SyncInfo(
                    on_wait=kept, on_update=list(si.on_update)
                )
                si = inst.sync_info
                wclocks = [wclocks[i] for i in best]
                waits = kept

        comp = dict(base)
        for c in wclocks:
            if c is not None:
                join(comp, c)

        if si:
            for u in si.on_update:
                if u.sync_type == "semaphore" and u.update_mode in ("sem-inc", "sem-add-imm"):
                    cum = sem_cum.get(u.id, 0) + (u.update_value or 0)
                    sem_cum[u.id] = cum
                    uclk = dict(comp)
                    uclk[u.id] = cum
                    sem_hist.setdefault(u.id, []).append((cum, uclk))

        if eng in _SERIAL_ENGINES:
            if is_dma:
                queue_clock[eng] = base
            else:
                qc = comp
                if si:
                    for u in si.on_update:
                        if (
                            u.sync_type == "semaphore"
                            and u.update_mode in ("sem-inc", "sem-add-imm")
                        ):
                            qc[u.id] = max(qc.get(u.id, 0), sem_cum[u.id])
                queue_clock[eng] = qc

    for inst in insts:
        si = inst.sync_info
        if type(inst).__name__ in _CHECKED_OPS and si and len(si.on_wait) > 1:
            raise RuntimeError(
                f"{inst.name} ({type(inst).__name__}) still has "
                f"{len(si.on_wait)} waits: "
                f"{[(w.ant_name, w.wait_value) for w in si.on_wait]}"
            )


def _build_nc(repeat=1):
    import concourse.bass as bass
    import concourse.tile as tile
    from concourse import mybir

    nc = bass.Bass(trn_type="TRN2")
    bf16 = mybir.dt.bfloat16
    f32 = mybir.dt.float32

    rhs_d = nc.dram_tensor("rhs", [2, 128, TWO_N], bf16, kind="ExternalInput")
    lhs_d = nc.dram_tensor("lhsT", [2, 128, SLAB], bf16, kind="ExternalInput")
    s_d = nc.dram_tensor("s_out", [128, M_TILES * N_GROUPS], f32,
                         kind="ExternalOutput")

    with tile.TileContext(nc) as tc:
        with (
            tc.tile_pool(name="data", bufs=1) as data,
            tc.tile_pool(name="psum", bufs=2, space=bass.MemorySpace.PSUM) as psum,
            tc.tile_pool(name="scr", bufs=2) as scrp,
            tc.tile_pool(name="outp", bufs=1) as outp,
        ):
            lhs_sb = [
                data.tile([128, SLAB], bf16, name=f"lhs{h}") for h in range(2)
            ]
            rhs_sb = [
                data.tile([128, TWO_N], bf16, name=f"rhs{h}") for h in range(2)
            ]
            s_sb = outp.tile([128, M_TILES * N_GROUPS], f32, name="s_sb")

            for h in range(2):
                nc.sync.dma_start(lhs_sb[h][:], lhs_d[h])
            for h in range(2):
                nc.sync.dma_start(rhs_sb[h][:], rhs_d[h])

            for _rep in range(repeat):
                for m in range(M_TILES):
                    for g in range(N_GROUPS):
                        ps = psum.tile([128, GROUP], f32, name="ps")
                        for h in range(2):
                            lw = lhs_sb[h][:, m * 128 : (m + 1) * 128]
                            for c in range(4):
                                col = g * GROUP + c * N_CHUNK
                                nc.tensor.matmul(
                                    ps[:, c * N_CHUNK : (c + 1) * N_CHUNK],
                                    lw,
                                    rhs_sb[h][:, col : col + N_CHUNK],
                                    start=(h == 0),
                                    stop=(h == 1),
                                )
                        scr = scrp.tile([128, GROUP], f32, name="scr")
                        mg = m * N_GROUPS + g
                        nc.scalar.activation(
                            scr[:],
                            ps[:],
                            mybir.ActivationFunctionType.Exp,
                            scale=2.0,
                            accum_out=s_sb[:, mg : mg + 1],
                        )
            nc.sync.dma_start(s_d[:], s_sb[:])

    nc.finalize()
    _reduce_waits(nc, tc)
    return nc


def _get_nc(repeat=1):
    if repeat not in _cached_nc:
        _cached_nc[repeat] = _build_nc(repeat)
    return _cached_nc[repeat]


def _prep_inputs(z_i, z_j):
    zi = np.asarray(z_i, dtype=np.float32).astype(np.float64)
    zj = np.asarray(z_j, dtype=np.float32).astype(np.float64)

    zin = zi / np.maximum(np.sqrt((zi * zi).sum(1, keepdims=True)), 1e-12)
    zjn = zj / np.maximum(np.sqrt((zj * zj).sum(1, keepdims=True)), 1e-12)

    reps = np.concatenate([zin, zjn], axis=0)  # [2N, D] float64, unit rows
    ztn_bf = reps.T.astype(ml_dtypes.bfloat16)  # [D, 2N] — what the PE sees
    rhs_np = np.ascontiguousarray(ztn_bf.reshape(2, 128, TWO_N))

    in_maps = []
    for c in range(N_CORES):
        lhs_np = np.ascontiguousarray(rhs_np[:, :, c * SLAB : (c + 1) * SLAB])
        in_maps.append({"rhs": rhs_np, "lhsT": lhs_np})
    return zin, zjn, ztn_bf, in_maps


def kernel(z_i: np.ndarray, z_j: np.ndarray) -> np.ndarray:
    global LAST_EXEC_NS, LAST_RESULTS
    from concourse.bass_utils import run_bass_kernel_spmd

    zin, zjn, ztn_bf, in_maps = _prep_inputs(z_i, z_j)

    nc = _get_nc()
    res = run_bass_kernel_spmd(nc, in_maps, core_ids=list(range(N_CORES)))
    LAST_EXEC_NS = res.exec_time_ns
    LAST_RESULTS = res

    s_full = np.empty(TWO_N, dtype=np.float64)
    for c in range(N_CORES):
        out = np.asarray(res.results[c]["s_out"], dtype=np.float64)  # [128,32]
        # column m*4+g holds the group-g partial for row tile m
        per_m = out.reshape(128, M_TILES, N_GROUPS).sum(axis=2)  # [128, 8]
        s_full[c * SLAB : (c + 1) * SLAB] = per_m.T.reshape(SLAB)

    # exact diagonal term as the PE computed it (bf16 inputs, fp32 accum)
    bf64 = ztn_bf.astype(np.float64)  # [D, 2N]
    rr = (bf64 * bf64).sum(axis=0)  # [2N]

    pos = (zin * zjn).sum(axis=1)  # [N]
    pos_full = np.concatenate([pos, pos])  # [2N]

    denom = s_full - np.exp(2.0 * rr)
    loss_rows = np.log(denom) - 2.0 * pos_full
    loss = loss_rows.mean()
    return np.asarray(loss, dtype=np.float32)
